# revision 1
# baseline (speedup 1.0000x reference)
"""Trainium2 Bass kernel for the dual-stack LSTM (A2LM_LMAudioPrev).

Strategy: 6-stage layer pipeline across cores 0-5 (a0,a1,a2,l0,l1,l2+head),
blocked wavefront (S=8 steps per block), one 8-wide AllGather per tick with a
1-tick consume lag, bf16 matmuls with f32 accumulation.  All rank-dependence
is delivered via per-core input data (weights, shifted x slices, one-hot
predecessor masks); the instruction graph is identical on all 8 cores.

Per-core cell (gates^T layout, partition = gate row, free = batch):
  gates_t = W_ihb @ pred_h_t  (batched per block)  +  P_t (x-projection,
  precomputed on-device just-in-time)  +  b  +  W_hh @ h_{t-1}  (serial)
"""
import numpy as np
import ml_dtypes

import concourse.bass as bass
import concourse.tile as tile
from concourse import bacc, mybir
from concourse.bass_utils import run_bass_kernel_spmd
BF16 = ml_dtypes.bfloat16
BF = mybir.dt.bfloat16
F32 = mybir.dt.float32

B = 64          # batch
H = 512         # hidden
D = 336         # audio feature dim
GH = 4 * H      # 2048 gate rows
S = 8           # steps per block
KH = 4          # k-tiles of hidden (512/128)
KP = 3          # k-tiles of padded x (384/128)
MT = 16         # m-tiles of gate rows (2048/128)
MHD = 2         # m-tiles of (padded) head output
NC = 8
FILL = 10       # 2 ticks of lag per stage x 5 stages


def _sigmoid(x):
    return 1.0 / (1.0 + np.exp(-x))


def _np_cell(x, h, c, Wih, Whh, b):
    g = x @ Wih.T + h @ Whh.T + b
    i, f, gg, o = np.split(g, 4, axis=-1)
    c2 = _sigmoid(f) * c + _sigmoid(i) * np.tanh(gg)
    h2 = _sigmoid(o) * np.tanh(c2)
    return h2, c2


def _build(nticks):
    nc = bacc.Bacc("TRN2", target_bir_lowering=False, debug=False, num_devices=NC)

    # --- per-core external inputs (bf16 unless noted) ---
    whhT = nc.dram_tensor("whhT", [KH, 128, GH], BF, kind="ExternalInput")
    wibT = nc.dram_tensor("wibT", [KH, 128, GH], BF, kind="ExternalInput")
    wpreT = nc.dram_tensor("wpreT", [KP, 128, GH], BF, kind="ExternalInput")
    weffT = nc.dram_tensor("weffT", [KH, 128, MHD * 128], BF, kind="ExternalInput")
    bias_d = nc.dram_tensor("bias", [128, MT], F32, kind="ExternalInput")
    beff_d = nc.dram_tensor("beff", [128, MHD], F32, kind="ExternalInput")
    h0T_d = nc.dram_tensor("h0T", [128, KH, B], BF, kind="ExternalInput")
    c0T_d = nc.dram_tensor("c0T", [128, KH, B], F32, kind="ExternalInput")
    xpre_d = nc.dram_tensor("xpre", [nticks, KP, 128, S * B], BF, kind="ExternalInput")
    mask_d = nc.dram_tensor("mask", [128, NC], F32, kind="ExternalInput")
    kill_d = nc.dram_tensor("kill", [128, 2 * FILL], F32, kind="ExternalInput")

    outT = nc.dram_tensor("outT", [nticks, S, MHD, 128, B], F32, kind="ExternalOutput")

    # --- internal DRAM (collective bounce) ---
    agin = nc.dram_tensor("agin", [128, KH * S * B], BF)
    agout = [nc.dram_tensor(f"agout{i}", [NC * 128, KH * S * B], BF, addr_space="Shared")
             for i in range(2)]

    SB = S * B  # 512 columns per block per k-tile

    with tile.TileContext(nc) as tc:
        with (
            tc.tile_pool(name="wpool", bufs=1) as wpool,
            tc.tile_pool(name="state", bufs=1) as state,
            tc.tile_pool(name="obufp", bufs=1) as obufp,
            tc.tile_pool(name="ppool", bufs=1) as ppool,
            tc.tile_pool(name="precp", bufs=1) as precp,
            tc.tile_pool(name="slotp", bufs=1) as slotp,
            tc.tile_pool(name="xpp", bufs=2) as xpp,
            tc.tile_pool(name="gpool", bufs=2) as gpool,
            tc.tile_pool(name="tmp", bufs=2) as tmpp,
            tc.tile_pool(name="hout", bufs=2) as houtp,
            tc.tile_pool(name="psg", bufs=2, space="PSUM") as psg,
            tc.tile_pool(name="psb", bufs=2, space="PSUM") as psb,
            tc.tile_pool(name="psh", bufs=2, space="PSUM") as psh,
        ):
            # ---- persistent SBUF ----
            whh_sb = wpool.tile([128, KH * GH], BF, tag="whh")
            wib_sb = wpool.tile([128, KH * GH], BF, tag="wib")
            wpre_sb = wpool.tile([128, KP * GH], BF, tag="wpre")
            weff_sb = wpool.tile([128, KH * MHD * 128], BF, tag="weff")
            bias_sb = wpool.tile([128, MT], F32, tag="bias")
            beff_sb = wpool.tile([128, MHD], F32, tag="beff")
            mask_sb = wpool.tile([128, NC], F32, tag="mask")
            for k in range(KH):
                nc.sync.dma_start(whh_sb[:, k * GH:(k + 1) * GH], whhT[k])
                nc.sync.dma_start(wib_sb[:, k * GH:(k + 1) * GH], wibT[k])
                nc.sync.dma_start(weff_sb[:, k * MHD * 128:(k + 1) * MHD * 128], weffT[k])
            for k in range(KP):
                nc.sync.dma_start(wpre_sb[:, k * GH:(k + 1) * GH], wpreT[k])
            nc.sync.dma_start(bias_sb[:], bias_d[:, :])
            nc.sync.dma_start(beff_sb[:], beff_d[:, :])
            nc.sync.dma_start(mask_sb[:], mask_d[:, :])
            kill_sb = wpool.tile([128, 2 * FILL], F32, tag="kill")
            nc.sync.dma_start(kill_sb[:], kill_d[:, :])
            h0_sb = wpool.tile([128, KH * B], BF, tag="h0sb")
            c0_sb = wpool.tile([128, KH * B], F32, tag="c0sb")
            nc.sync.dma_start(h0_sb[:], h0T_d[:, :, :])
            nc.sync.dma_start(c0_sb[:], c0T_d[:, :, :])

            cT = state.tile([128, KH * B], F32, tag="cT")        # cell state (f32)
            nc.vector.tensor_copy(cT[:], c0_sb[:])

            # h blocks, double buffered; h0 preloaded in last slot of obuf[1]
            obuf = [obufp.tile([128, KH, SB], BF, tag=f"ob{i}", name=f"ob{i}") for i in range(2)]
            nc.vector.memset(obuf[1][:], 0.0)
            nc.sync.dma_start(obuf[1][:, :, (S - 1) * B:], h0T_d[:, :, :])

            P_sb = [ppool.tile([128, S, MT * B], BF, tag=f"P{i}", name=f"P{i}") for i in range(2)]
            prec = [precp.tile([128, S, MT * B], BF, tag=f"pc{i}", name=f"pc{i}") for i in range(2)]
            slots = [slotp.tile([128, KH * SB], BF, tag=f"sl{r}", name=f"sl{r}") for r in range(NC)]
            pred = [slotp.tile([128, KH, SB], BF, tag=f"pr{i}", name=f"pr{i}") for i in range(2)]
            nc.vector.memset(pred[0][:], 0.0)
            nc.vector.memset(pred[1][:], 0.0)

            xp_tiles = {}

            def pjit_dma(bb):
                xp = xpp.tile([128, KP, SB], BF, tag="xp", name=f"xp{bb}")
                xp_tiles[bb] = xp
                for k in range(KP):
                    nc.sync.dma_start(xp[:, k], xpre_d[bb, k])

            def pjit_part(bb, m0, m1):
                """compute P[bb] m-tiles [m0,m1) into P_sb[bb % 2]."""
                xp = xp_tiles[bb]
                for m in range(m0, m1):
                    ps = psb.tile([128, SB], F32, tag="psb", name=f"psp{bb}_{m}")
                    for k in range(KP):
                        nc.tensor.matmul(
                            ps[:], wpre_sb[:, k * GH + m * 128: k * GH + (m + 1) * 128],
                            xp[:, k], start=(k == 0), stop=(k == KP - 1))
                    nc.vector.tensor_copy(
                        P_sb[bb % 2][:, :, m * B:(m + 1) * B],
                        ps[:].rearrange("p (s b) -> p s b", s=S))

            def batch_part(bb, m0, m1):
                """precomp[bb] m-tiles [m0,m1): W_ihb @ pred + bias + P[bb]."""
                for m in range(m0, m1):
                    ps = psb.tile([128, SB], F32, tag="psb", name=f"psb{bb}_{m}")
                    for k in range(KH):
                        nc.tensor.matmul(
                            ps[:], wib_sb[:, k * GH + m * 128: k * GH + (m + 1) * 128],
                            pred[bb % 2][:, k], start=(k == 0), stop=(k == KH - 1))
                    t = tmpp.tile([128, SB], F32, tag="bt", name=f"bt{bb}_{m}")
                    nc.scalar.activation(t[:], ps[:],
                                         mybir.ActivationFunctionType.Identity,
                                         bias=bias_sb[:, m:m + 1])
                    nc.vector.tensor_tensor(
                        prec[bb % 2][:, :, m * B:(m + 1) * B],
                        t[:].rearrange("p (s b) -> p s b", s=S),
                        P_sb[bb % 2][:, :, m * B:(m + 1) * B],
                        mybir.AluOpType.add)

            def comm(bb):
                """AllGather obuf[bb]; masked-select predecessor into pred[bb % 2]."""
                nc.sync.dma_start(agin[:, :], obuf[bb % 2][:])
                nc.gpsimd.collective_compute(
                    "AllGather", mybir.AluOpType.bypass,
                    replica_groups=[list(range(NC))],
                    ins=[agin.ap().opt()], outs=[agout[bb % 2].ap().opt()])
                for r in range(NC):
                    nc.sync.dma_start(slots[r][:], agout[bb % 2][r * 128:(r + 1) * 128, :])
                dst = pred[bb % 2][:].rearrange("p k n -> p (k n)")
                nc.vector.tensor_scalar_mul(dst, slots[0][:], mask_sb[:, 0:1])
                for r in range(1, NC):
                    nc.vector.scalar_tensor_tensor(
                        dst, slots[r][:], mask_sb[:, r:r + 1], dst,
                        mybir.AluOpType.mult, mybir.AluOpType.add)

            def steps(bb, nticks=None):
                for s in range(S):
                    if s == 0:
                        hsrc = obuf[(bb - 1) % 2][:, :, (S - 1) * B:]
                    else:
                        hsrc = obuf[bb % 2][:, :, (s - 1) * B: s * B]
                    psum_g = psg.tile([128, MT * B], F32, tag="pg")
                    for m in range(MT):
                        for k in range(KH):
                            nc.tensor.matmul(
                                psum_g[:, m * B:(m + 1) * B],
                                whh_sb[:, k * GH + m * 128: k * GH + (m + 1) * 128],
                                hsrc[:, k], start=(k == 0), stop=(k == KH - 1))
                    # fill PE during the elementwise window.  x-projection chunks
                    # (no comm dependency) go early in the tick; batch chunks
                    # (need last tick's AllGather+select) go late.
                    if s < 4:
                        if bb + 2 < NT[0]:
                            pjit_part(bb + 2, 4 * s, 4 * s + 4)
                    else:
                        if bb + 1 < NT[0]:
                            batch_part(bb + 1, 4 * (s - 4), 4 * (s - 4) + 4)
                    g = gpool.tile([128, MT * B], F32, tag="g")
                    nc.vector.tensor_tensor(g[:], psum_g[:], prec[bb % 2][:, s],
                                            mybir.AluOpType.add)
                    IF, GG, OO = 8 * B, 12 * B, 16 * B
                    sif = tmpp.tile([128, IF], F32, tag="sif")
                    nc.scalar.activation(sif[:], g[:, 0:IF],
                                         mybir.ActivationFunctionType.Sigmoid)
                    tg = tmpp.tile([128, KH * B], F32, tag="tg")
                    nc.scalar.activation(tg[:], g[:, IF:GG],
                                         mybir.ActivationFunctionType.Tanh)
                    so = tmpp.tile([128, KH * B], F32, tag="so")
                    nc.scalar.activation(so[:], g[:, GG:OO],
                                         mybir.ActivationFunctionType.Sigmoid)
                    t1 = tmpp.tile([128, KH * B], F32, tag="t1")
                    nc.vector.tensor_tensor(t1[:], sif[:, KH * B:], cT[:],
                                            mybir.AluOpType.mult)
                    t2 = tmpp.tile([128, KH * B], F32, tag="t2")
                    nc.vector.tensor_tensor(t2[:], sif[:, 0:KH * B], tg[:],
                                            mybir.AluOpType.mult)
                    nc.vector.tensor_tensor(cT[:], t1[:], t2[:], mybir.AluOpType.add)
                    tc_ = tmpp.tile([128, KH * B], F32, tag="tc")
                    nc.scalar.activation(tc_[:], cT[:],
                                         mybir.ActivationFunctionType.Tanh)
                    hdst = obuf[bb % 2][:, :, s * B:(s + 1) * B]
                    nc.vector.tensor_tensor(
                        hdst, so[:].rearrange("p (k n) -> p k n", k=KH),
                        tc_[:].rearrange("p (k n) -> p k n", k=KH),
                        mybir.AluOpType.mult)
                    # head
                    ph = psh.tile([128, MHD * B], F32, tag="ph")
                    for mh in range(MHD):
                        for k in range(KH):
                            nc.tensor.matmul(
                                ph[:, mh * B:(mh + 1) * B],
                                weff_sb[:, k * MHD * 128 + mh * 128:
                                        k * MHD * 128 + (mh + 1) * 128],
                                obuf[bb % 2][:, k, s * B:(s + 1) * B],
                                start=(k == 0), stop=(k == KH - 1))
                    ho = houtp.tile([128, MHD * B], F32, tag="ho")
                    for mh in range(MHD):
                        nc.scalar.activation(ho[:, mh * B:(mh + 1) * B],
                                             ph[:, mh * B:(mh + 1) * B],
                                             mybir.ActivationFunctionType.Identity,
                                             bias=beff_sb[:, mh:mh + 1])
                    nc.sync.dma_start(
                        outT[bb, s].rearrange("m p b -> p m b"),
                        ho[:].rearrange("p (m b) -> p m b", m=MHD))

            def merge(bb):
                """fill-phase state reset: state = state*kA + init*kB (per-core data)."""
                kA, kB = kill_sb[:, 2 * bb:2 * bb + 1], kill_sb[:, 2 * bb + 1:2 * bb + 2]
                t = tmpp.tile([128, KH * B], F32, tag="mgc")
                nc.vector.tensor_scalar_mul(t[:], c0_sb[:], kB)
                nc.vector.scalar_tensor_tensor(cT[:], cT[:], kA, t[:],
                                               mybir.AluOpType.mult, mybir.AluOpType.add)
                th = tmpp.tile([128, KH * B], BF, tag="mgh")
                nc.vector.tensor_scalar_mul(th[:], h0_sb[:], kB)
                hlast = obuf[bb % 2][:, :, (S - 1) * B:]
                nc.vector.scalar_tensor_tensor(
                    hlast, hlast, kA,
                    th[:].rearrange("p (k n) -> p k n", k=KH),
                    mybir.AluOpType.mult, mybir.AluOpType.add)

            # ---- preamble: P[0], P[1], precomp[0] ----
            NT = [nticks]
            pjit_dma(0)
            pjit_part(0, 0, MT)
            pjit_dma(1)
            pjit_part(1, 0, MT)
            batch_part(0, 0, MT)
            # ---- main loop ----
            for bb in range(nticks):
                if bb + 2 < nticks:
                    pjit_dma(bb + 2)
                steps(bb)
                if bb < FILL:
                    merge(bb)
                if bb + 2 < nticks:
                    comm(bb)
    nc.compile()
    return nc


_NC_CACHE = {}


def _get_nc(nticks):
    if nticks not in _NC_CACHE:
        _NC_CACHE[nticks] = _build(nticks)
    return _NC_CACHE[nticks]


def _wT_layout(W, ktiles):
    """W [GH, IN(pad to 128*ktiles)] -> [ktiles, 128, GH] bf16 (lhsT tiles)."""
    gh, inw = W.shape
    Wp = np.zeros((gh, ktiles * 128), np.float32)
    Wp[:, :inw] = W
    return np.ascontiguousarray(Wp.T.reshape(ktiles, 128, gh)).astype(BF16)


def _hT_layout(h):
    """h [B, H] -> [128, KH, B]"""
    return np.ascontiguousarray(h.T.reshape(KH, 128, B).transpose(1, 0, 2))


def kernel(**inputs):
    audio = np.asarray(inputs["audio"], np.float32)
    T = audio.shape[1]
    nblk = (T + S - 1) // S
    nticks = nblk + FILL

    f = {k: np.asarray(v, np.float32) for k, v in inputs.items() if not k.startswith("_")}

    # ---- host prelude (frame-0 audio_step, t=0 lm step, head collapse) ----
    x0 = audio[:, 0, :]
    ha, ca = [f["h0a"]] * 3, [f["c0a"]] * 3
    h, c = _np_cell(x0, ha[0], ca[0], f["aW_ih0"], f["aW_hh0"], f["ab0"])
    ahs, acs = [h], [c]
    for i in range(2):
        h, c = _np_cell(ahs[-1], ha[i + 1], ca[i + 1], f["aW_ih"][i], f["aW_hh"][i], f["ab"][i])
        ahs.append(h); acs.append(c)
    aprev0 = ahs[-1]
    W_eff = f["fc2_w"] @ f["fc1_w"]                      # [136, 512]
    b_eff = f["fc1_b"] @ f["fc2_w"].T + f["fc2_b"]       # [136]
    lm_in0 = np.concatenate([aprev0, x0], axis=1)
    lh, lc = [f["h0l"]] * 3, [f["c0l"]] * 3
    h, c = _np_cell(lm_in0, lh[0], lc[0], f["lW_ih0"], f["lW_hh0"], f["lb0"])
    lhs_, lcs = [h], [c]
    for i in range(2):
        h, c = _np_cell(lhs_[-1], lh[i + 1], lc[i + 1], f["lW_ih"][i], f["lW_hh"][i], f["lb"][i])
        lhs_.append(h); lcs.append(c)
    out0 = lhs_[-1] @ W_eff.T + b_eff                    # [B, 136]

    # ---- per-core data ----
    Z_GH_H = np.zeros((GH, H), np.float32)
    Z_GH = np.zeros((GH,), np.float32)
    Wpre_x = [f["aW_ih0"], None, None, f["lW_ih0"][:, H:], None, None, None, None]
    Whh_l = [f["aW_hh0"], f["aW_hh"][0], f["aW_hh"][1],
             f["lW_hh0"], f["lW_hh"][0], f["lW_hh"][1], Z_GH_H, Z_GH_H]
    Wib_l = [Z_GH_H, f["aW_ih"][0], f["aW_ih"][1],
             f["lW_ih0"][:, :H], f["lW_ih"][0], f["lW_ih"][1], Z_GH_H, Z_GH_H]
    bias_l = [f["ab0"], f["ab"][0], f["ab"][1], f["lb0"], f["lb"][0], f["lb"][1], Z_GH, Z_GH]
    h0_l = [ahs[0], ahs[1], ahs[2], lhs_[0], lhs_[1], lhs_[2],
            np.zeros((B, H), np.float32), np.zeros((B, H), np.float32)]
    c0_l = [acs[0], acs[1], acs[2], lcs[0], lcs[1], lcs[2],
            np.zeros((B, H), np.float32), np.zeros((B, H), np.float32)]
    W_eff_pad = np.zeros((MHD * 128, H), np.float32)
    W_eff_pad[:136] = W_eff
    b_eff_pad = np.zeros((MHD * 128,), np.float32)
    b_eff_pad[:136] = b_eff

    # xpre slabs: [nticks, KP, 128, S*B]
    xpadT = np.zeros((KP * 128, T, B), np.float32)   # [feat, t, b]
    xpadT[:D, :, :] = audio.transpose(2, 1, 0)
    def xpre_for(core):
        out = np.zeros((nticks, KP, 128, S * B), np.float32)
        if core == 0:
            lag, t0 = 0, 0
        elif core == 3:
            lag, t0 = 6, 1
        else:
            return out.astype(BF16)
        for bb in range(nticks):
            g = bb - lag
            if not (0 <= g < nblk):
                continue
            for s in range(S):
                t = S * g + t0 + s
                if t >= T:
                    continue
                blk = xpadT[:, t, :]  # [KP*128, B]
                out[bb, :, :, s * B:(s + 1) * B] = blk.reshape(KP, 128, B)
        return out.astype(BF16)

    pred_of = [7, 0, 1, 2, 3, 4, 5, 6]
    in_maps = []
    for r in range(NC):
        mask = np.zeros((128, NC), np.float32)
        mask[:, pred_of[r]] = 1.0
        kill = np.zeros((128, 2 * FILL), np.float32)
        for bbk in range(FILL):
            keep = 1.0 if bbk >= 2 * r else 0.0
            kill[:, 2 * bbk] = keep
            kill[:, 2 * bbk + 1] = 1.0 - keep
        wpre = Wpre_x[r] if Wpre_x[r] is not None else np.zeros((GH, D), np.float32)
        in_maps.append({
            "whhT": _wT_layout(Whh_l[r], KH),
            "wibT": _wT_layout(Wib_l[r], KH),
            "wpreT": _wT_layout(wpre, KP),
            "weffT": (_wT_layout(W_eff_pad, KH) if r == 5
                      else np.zeros((KH, 128, MHD * 128), BF16)),
            "bias": np.ascontiguousarray(bias_l[r].reshape(MT, 128).T).astype(np.float32),
            "beff": (np.ascontiguousarray(b_eff_pad.reshape(MHD, 128).T).astype(np.float32)
                     if r == 5 else np.zeros((128, MHD), np.float32)),
            "h0T": _hT_layout(h0_l[r]).astype(BF16),
            "c0T": _hT_layout(c0_l[r]).astype(np.float32),
            "xpre": xpre_for(r),
            "mask": mask.astype(np.float32),
            "kill": kill,
        })

    nc = _get_nc(nticks)
    trace = bool(inputs.get("_trace", False))
    res = run_bass_kernel_spmd(nc, in_maps, core_ids=list(range(NC)), trace=trace)

    outT = res.results[5]["outT"]  # [nticks, S, MHD, 128, B]
    out = np.zeros((B, T, 136), np.float32)
    out[:, 0, :] = out0
    for t in range(1, T):
        g, s = (t - 1) // S, (t - 1) % S
        blk = outT[g + FILL, s]          # [MHD, 128, B]
        out[:, t, :] = blk.reshape(MHD * 128, B)[:136].T
    if trace:
        kernel._last = res
    return out

